# revision 13
# baseline (speedup 1.0000x reference)
"""Multi-head causal attention with RoPE on 8 trn2 cores — v2.

Sharding: core c -> batch b = c // 4, head group g = c % 4 (heads 4g..4g+4).
Each core computes q/k/v projections for its 4 heads, causal attention, and
a partial output-projection (its heads' slice of Wo). The host sums the 4
bf16 partials per batch and adds the output bias.

v2 changes vs v1 (273us baseline):
  - All projections are weight-stationary with n-pair grouping (each
    LDWEIGHTS feeds 2 matmuls); v is projected transposed like q/k and
    moved to [s, d] PV layout with DMA-xbar transposes off the PE.
  - Biases fold into the matmuls via a bias row x ones row; when all
    biases are zero (the bench case) that path compiles out entirely.
  - Emission is software-pipelined for the scarce engines: the exp-feeding
    chain (projections n01 -> scores/exp m0/m1 -> projections n23 ->
    scores/exp m2/m3) is emitted ahead of the PV/finalize/out-projection
    chain, which fills PE gaps; keeps ACT (exp ~64us floor) saturated and
    the PE dense enough that HAM stays at K=8/8 (v1 ran attention at
    half-clock for 153us).
  - Input DMAs ordered so the first matmul starts ~2us in; output is bf16,
    written per [128,512] chunk as the out-projection drains.
"""

import os

import numpy as np
import ml_dtypes

BF16 = ml_dtypes.bfloat16

B, S, F = 2, 2048, 1024
H, D = 16, 64
HALF = D // 2
NCORES = 8
HPC = 4  # heads per core
S_TILES = S // 128  # 16
N_CH = S // 512  # 4
F_CH = F // 128  # 8
MACROS = 4
MAX_WAVELENGTH = 10000.0

_CACHE = {}
LAST_RESULT = None


def _build_nc(with_bias):
    import concourse.bacc as bacc
    import concourse.tile as tile
    import concourse.mybir as mybir

    fp32 = mybir.dt.float32
    bf16 = mybir.dt.bfloat16
    MULT = mybir.AluOpType.mult
    EXP = mybir.ActivationFunctionType.Exp

    nc = bacc.Bacc("TRN2", target_bir_lowering=False, debug=False)

    xT_d = nc.dram_tensor("xT", [F + 1, S], bf16, kind="ExternalInput")
    wq_d = nc.dram_tensor("wq", [F + 1, 256], bf16, kind="ExternalInput")
    wk_d = nc.dram_tensor("wk", [F + 1, 256], bf16, kind="ExternalInput")
    wv_d = nc.dram_tensor("wv", [F + 1, 256], bf16, kind="ExternalInput")
    wo_d = nc.dram_tensor("wo", [256, F], bf16, kind="ExternalInput")
    cos_d = nc.dram_tensor("cosw", [32, S], bf16, kind="ExternalInput")
    sin_d = nc.dram_tensor("sinw", [32, S], bf16, kind="ExternalInput")
    mask_d = nc.dram_tensor("mask", [128, 256], bf16, kind="ExternalInput")
    outT_d = nc.dram_tensor("outT", [F, S], bf16, kind="ExternalOutput")

    with tile.TileContext(nc) as tc:
        with (
            tc.tile_pool(name="persist", bufs=1) as persist,
            tc.tile_pool(name="tmp", bufs=12) as tmp,
            tc.tile_pool(name="attn", bufs=30 if not with_bias else 26) as attn_pool,
            tc.tile_pool(name="fin", bufs=3) as fin,
            tc.tile_pool(name="psProj", bufs=2, space="PSUM") as psProj,
            tc.tile_pool(name="psSps", bufs=2, space="PSUM") as psSps,
            tc.tile_pool(name="psPV", bufs=2, space="PSUM") as psPV,
        ):
            # ---------------- persistent tiles + ordered input DMAs -----
            # order = first-use order: wq+xT gate the first matmul, wk the
            # second projection, cos/sin the RoPE, wv/wo come later
            cosw = persist.tile([128, S], bf16, tag="cosw", name="cosw")
            sinw = persist.tile([128, S], bf16, tag="sinw", name="sinw")
            maskt = persist.tile([128, 256], bf16, tag="maskt", name="maskt")
            wq = [persist.tile([128, 256], bf16, tag=f"wq{i}", name=f"wq{i}") for i in range(F_CH)]
            xT = [persist.tile([128, S], bf16, tag=f"xT{i}", name=f"xT{i}") for i in range(F_CH)]
            wk = [persist.tile([128, 256], bf16, tag=f"wk{i}", name=f"wk{i}") for i in range(F_CH)]
            wv = [persist.tile([128, 256], bf16, tag=f"wv{i}", name=f"wv{i}") for i in range(F_CH)]
            wo = [persist.tile([128, F], bf16, tag=f"wo{i}", name=f"wo{i}") for i in range(2)]
            for i in range(F_CH):
                nc.sync.dma_start(out=wq[i], in_=wq_d[128 * i : 128 * (i + 1), :])
            for i in range(F_CH):
                nc.sync.dma_start(out=wk[i], in_=wk_d[128 * i : 128 * (i + 1), :])
            for i in range(F_CH):
                nc.sync.dma_start(out=xT[i], in_=xT_d[128 * i : 128 * (i + 1), :])
            # quarter-size rope tables (the 4 head-copies are identical):
            # HBM ships 32 rows, SBUF-to-SBUF hops replicate to 128
            nc.sync.dma_start(out=cosw[0:32, :], in_=cos_d[0:32, :])
            nc.sync.dma_start(out=sinw[0:32, :], in_=sin_d[0:32, :])
            nc.sync.dma_start(out=maskt, in_=mask_d[:, :])
            for r in range(1, 4):
                nc.sync.dma_start(out=cosw[32 * r : 32 * (r + 1), :], in_=cosw[0:32, :])
                nc.sync.dma_start(out=sinw[32 * r : 32 * (r + 1), :], in_=sinw[0:32, :])
            for i in range(F_CH):
                nc.sync.dma_start(out=wv[i], in_=wv_d[128 * i : 128 * (i + 1), :])
            for i in range(2):
                nc.sync.dma_start(out=wo[i], in_=wo_d[128 * i : 128 * (i + 1), :])

            if with_bias:
                xones = persist.tile([1, S], bf16, tag="xones", name="xones")
                wqb = persist.tile([1, 256], bf16, tag="wqb", name="wqb")
                wkb = persist.tile([1, 256], bf16, tag="wkb", name="wkb")
                wvb = persist.tile([1, 256], bf16, tag="wvb", name="wvb")
                nc.sync.dma_start(out=xones, in_=xT_d[F : F + 1, :])
                nc.sync.dma_start(out=wqb, in_=wq_d[F : F + 1, :])
                nc.sync.dma_start(out=wkb, in_=wk_d[F : F + 1, :])
                nc.sync.dma_start(out=wvb, in_=wv_d[F : F + 1, :])
            else:
                xones = wqb = wkb = wvb = None

            # PE warm-up: ~20 no-dep matmuls flip the HAM clock gate to
            # K=8/8 (~3.4us of activity) before the real data arrives, so
            # the DMA-paced first projection runs at 2.4 GHz
            wdum = persist.tile([128, 128], bf16, tag="wdum", name="wdum")
            xdum = persist.tile([128, 512], bf16, tag="xdum", name="xdum")
            nc.vector.memset(wdum[:, :], 0.0)
            nc.vector.memset(xdum[:, :], 0.0)
            tdum = psProj.tile([128, 512], fp32, tag="proj", name="tdum")
            for _ in range(44):
                nc.tensor.matmul(tdum, wdum, xdum, start=True, stop=True)

            # post-RoPE q/k, [d, s]; chunk 1 = x1 halves of the 4 heads
            # (head h -> partitions 32h..32h+32), chunk 2 = x2 halves.
            q1 = persist.tile([128, S], bf16, tag="q1", name="q1")
            q2 = persist.tile([128, S], bf16, tag="q2", name="q2")
            k1 = persist.tile([128, S], bf16, tag="k1", name="k1")
            k2 = persist.tile([128, S], bf16, tag="k2", name="k2")
            # v projected transposed ([d, s], head-major rows), then moved
            # to [s, d] layout; head h cols 65h..65h+64, col 65h+64 = ones.
            vT = [persist.tile([128, S], bf16, tag=f"vT{i}", name=f"vT{i}") for i in range(2)]
            # v in [s, d] layout: per-head tile, s-chunk st at cols
            # 128*st (64 dims + denominator-ones col at 128*st+64); the
            # 128-col stride keeps every xbar-transpose run 256B-aligned
            v_sb = [persist.tile([128, S], bf16, tag=f"vs{h}", name=f"vs{h}")
                    for h in range(HPC)]
            # attention output, [dh, s] (head h -> tile h//2 rows 64*(h%2))
            aoT = [persist.tile([128, S], bf16, tag=f"aoT{i}", name=f"aoT{i}") for i in range(2)]

            # ones columns of v_sb (denominator accumulators); the v
            # transposes write disjoint columns so order doesn't matter
            for hl in range(HPC):
                ones_v = v_sb[hl][:, :].rearrange("p (t c) -> p t c", c=128)[:, :, 64:65]
                nc.gpsimd.memset(ones_v, 1.0)

            # ---------------- weight-stationary projections -------------
            def qk01_streamed():
                """q n0/n1 (both halves) + k n0/n1, paced by xT chunk
                arrival: q uses the attention-phase psSps banks (idle here),
                k halfA rides the same kc sweep through psProj, halfB
                follows once halfA has drained."""
                qA = psSps.tile([128, 1024], fp32, tag="sps", name="qA")
                qB = psSps.tile([128, 1024], fp32, tag="sps", name="qB")
                kA = [psProj.tile([128, 512], fp32, tag="proj", name=f"kA{j}") for j in range(2)]
                for kc in range(F_CH):
                    st = (kc == 0)
                    sp = (kc == F_CH - 1 and not with_bias)
                    for j in range(2):
                        nc.tensor.matmul(qA[:, 512 * j : 512 * (j + 1)], wq[kc][:, 0:128],
                                         xT[kc][:, 512 * j : 512 * (j + 1)], start=st, stop=sp)
                        nc.tensor.matmul(qB[:, 512 * j : 512 * (j + 1)], wq[kc][:, 128:256],
                                         xT[kc][:, 512 * j : 512 * (j + 1)], start=st, stop=sp)
                        nc.tensor.matmul(kA[j], wk[kc][:, 0:128],
                                         xT[kc][:, 512 * j : 512 * (j + 1)], start=st, stop=sp)
                if with_bias:
                    for j in range(2):
                        jsl = slice(512 * j, 512 * (j + 1))
                        nc.tensor.matmul(qA[:, jsl], wqb[0:1, 0:128], xones[:, jsl],
                                         start=False, stop=True)
                        nc.tensor.matmul(qB[:, jsl], wqb[0:1, 128:256], xones[:, jsl],
                                         start=False, stop=True)
                        nc.tensor.matmul(kA[j], wkb[0:1, 0:128], xones[:, jsl],
                                         start=False, stop=True)
                cq = [[None, None], [None, None]]
                ck = [[None, None], [None, None]]
                for j in range(2):
                    cq[0][j] = tmp.tile([128, 512], bf16, tag="rope", name="cq0")
                    nc.scalar.copy(cq[0][j], qA[:, 512 * j : 512 * (j + 1)])
                    cq[1][j] = tmp.tile([128, 512], bf16, tag="rope", name="cq1")
                    nc.scalar.copy(cq[1][j], qB[:, 512 * j : 512 * (j + 1)])
                    ck[0][j] = tmp.tile([128, 512], bf16, tag="rope", name="ck0")
                    nc.scalar.copy(ck[0][j], kA[j])
                kB = [psProj.tile([128, 512], fp32, tag="proj", name=f"kB{j}") for j in range(2)]
                for kc in range(F_CH):
                    for j in range(2):
                        nc.tensor.matmul(kB[j], wk[kc][:, 128:256],
                                         xT[kc][:, 512 * j : 512 * (j + 1)],
                                         start=(kc == 0),
                                         stop=(kc == F_CH - 1 and not with_bias))
                if with_bias:
                    for j in range(2):
                        nc.tensor.matmul(kB[j], wkb[0:1, 128:256],
                                         xones[:, 512 * j : 512 * (j + 1)],
                                         start=False, stop=True)
                for j in range(2):
                    ck[1][j] = tmp.tile([128, 512], bf16, tag="rope", name="ck1")
                    nc.scalar.copy(ck[1][j], kB[j])
                for cs, o1, o2 in ((cq, q1, q2), (ck, k1, k2)):
                    for j in range(2):
                        nsl = slice(512 * j, 512 * (j + 1))
                        c1, c2 = cs[0][j], cs[1][j]
                        t1 = tmp.tile([128, 512], bf16, tag="rope", name="t1")
                        t2 = tmp.tile([128, 512], bf16, tag="rope", name="t2")
                        t3 = tmp.tile([128, 512], bf16, tag="rope", name="t3")
                        t4 = tmp.tile([128, 512], bf16, tag="rope", name="t4")
                        nc.vector.tensor_mul(t1, c1, cosw[:, nsl])
                        nc.vector.tensor_mul(t2, c2, sinw[:, nsl])
                        nc.vector.tensor_mul(t3, c2, cosw[:, nsl])
                        nc.vector.tensor_mul(t4, c1, sinw[:, nsl])
                        nc.vector.tensor_sub(o1[:, nsl], t1, t2)
                        nc.vector.tensor_add(o2[:, nsl], t3, t4)

            def wproj(w_sb, wb, npair, drains):
                """One projection, s-columns [1024*npair, 1024*(npair+1)).
                drains(half, j, ps) consumes the psum tile for n=2*npair+j."""
                for half in range(2):
                    hsl = slice(128 * half, 128 * (half + 1))
                    ts = [psProj.tile([128, 512], fp32, tag="proj", name=f"pj{j}") for j in range(2)]
                    for kc in range(F_CH):
                        for j in range(2):
                            n = 2 * npair + j
                            nc.tensor.matmul(ts[j], w_sb[kc][:, hsl],
                                             xT[kc][:, 512 * n : 512 * (n + 1)],
                                             start=(kc == 0),
                                             stop=(kc == F_CH - 1 and not with_bias))
                    if with_bias:
                        for j in range(2):
                            n = 2 * npair + j
                            nc.tensor.matmul(ts[j], wb[0:1, hsl],
                                             xones[:, 512 * n : 512 * (n + 1)],
                                             start=False, stop=True)
                    for j in range(2):
                        drains(half, j, ts[j])

            def qk_proj(w_sb, wb, o1, o2, npair):
                cs = [[None, None], [None, None]]

                def drains(half, j, ps):
                    c = tmp.tile([128, 512], bf16, tag="rope", name="c")
                    nc.scalar.copy(c, ps)
                    cs[half][j] = c

                wproj(w_sb, wb, npair, drains)
                for j in range(2):
                    n = 2 * npair + j
                    nsl = slice(512 * n, 512 * (n + 1))
                    c1, c2 = cs[0][j], cs[1][j]
                    t1 = tmp.tile([128, 512], bf16, tag="rope", name="t1")
                    t2 = tmp.tile([128, 512], bf16, tag="rope", name="t2")
                    t3 = tmp.tile([128, 512], bf16, tag="rope", name="t3")
                    t4 = tmp.tile([128, 512], bf16, tag="rope", name="t4")
                    nc.vector.tensor_mul(t1, c1, cosw[:, nsl])
                    nc.vector.tensor_mul(t2, c2, sinw[:, nsl])
                    nc.vector.tensor_mul(t3, c2, cosw[:, nsl])
                    nc.vector.tensor_mul(t4, c1, sinw[:, nsl])
                    nc.vector.tensor_sub(o1[:, nsl], t1, t2)
                    nc.vector.tensor_add(o2[:, nsl], t3, t4)

            def v_proj(npair):
                def drains(half, j, ps):
                    n = 2 * npair + j
                    nc.vector.tensor_copy(vT[half][:, 512 * n : 512 * (n + 1)], ps)

                wproj(wv, wvb, npair, drains)

            def v_transpose(npair):
                # strided xbar transpose per (head, n-pair): [64, 1024] ->
                # [(128x8), 64] with the 8 s-chunks 128 cols apart; split by
                # n-pair so early PV chunks unblock before the full v lands
                nsl = slice(1024 * npair, 1024 * (npair + 1))
                for half in range(2):
                    for hh in range(2):
                        hl = 2 * half + hh
                        out_v = v_sb[hl][:, nsl].rearrange("p (t c) -> p t c", c=128)[:, :, 0:64]
                        nc.sync.dma_start_transpose(
                            out=out_v, in_=vT[half][64 * hh : 64 * (hh + 1), nsl])

            # ---------------- attention -------------------------------
            at_tiles = {}  # (m, kk) -> [at_p0, at_p1]

            def scores_exp(m):
                for kk in range(4 * m + 4):
                    t = kk - 4 * m
                    lo = max(0, t) * 128
                    ksl = slice(128 * kk, 128 * (kk + 1))
                    qsl = slice(512 * m + lo, 512 * (m + 1))
                    sps_l = [psSps.tile([128, 1024], fp32, tag="sps", name=f"sps{p}")
                             for p in range(2)]
                    # wave order: all 4 heads' k1 matmuls (4 concurrent
                    # row-groups), then the k2 accumulate wave
                    for qq, kx in ((q1, k1), (q2, k2)):
                        for p in range(2):
                            for hh in range(2):
                                h = 2 * p + hh
                                hp = slice(32 * h, 32 * (h + 1))
                                osl = slice(512 * hh + lo, 512 * hh + 512)
                                nc.tensor.matmul(sps_l[p][:, osl], kx[hp, ksl], qq[hp, qsl],
                                                 start=(kx is k1), stop=(kx is k2),
                                                 tile_position=(32 * h, 0))
                    atp = []
                    for p in range(2):
                        at = attn_pool.tile([128, 1024], bf16, tag="attn", name="at")
                        sps_v = sps_l[p][:, :].rearrange("a (h q) -> a h q", h=2)[:, :, lo:512]
                        at_v = at[:, :].rearrange("a (h q) -> a h q", h=2)[:, :, lo:512]
                        nc.scalar.activation(out=at_v, in_=sps_v, func=EXP, scale=0.125)
                        if t >= 0:
                            dv = at[:, :].rearrange("a (h q) -> a h q", h=2)[:, :, 128 * t : 128 * (t + 1)]
                            nc.vector.tensor_tensor(dv, dv, maskt, op=MULT)
                        atp.append(at)
                    at_tiles[(m, kk)] = atp

            def finalize(pvt, h, m):
                msl = slice(512 * m, 512 * (m + 1))
                cix, r0 = h // 2, 64 * (h % 2)
                s65 = fin.tile([65, 512], fp32, tag="s65", name="s65")
                nc.vector.tensor_copy(s65, pvt[0:65, :])
                rec0 = fin.tile([1, 512], fp32, tag="rec0", name="rec0")
                nc.sync.dma_start(out=rec0, in_=s65[64:65, :])
                rcp = fin.tile([1, 512], fp32, tag="rcp", name="rcp")
                nc.vector.reciprocal_approx_fast(rcp, rec0)
                rb = fin.tile([64, 512], fp32, tag="rb", name="rb")
                nc.gpsimd.partition_broadcast(rb, rcp[0:1, :])
                if r0 == 0:
                    nc.vector.tensor_tensor(aoT[cix][0:64, msl], s65[0:64, :], rb, op=MULT)
                else:
                    ao = fin.tile([64, 512], bf16, tag="ao", name="ao")
                    nc.vector.tensor_tensor(ao, s65[0:64, :], rb, op=MULT)
                    nc.sync.dma_start(out=aoT[cix][64:128, msl], in_=ao)

            def pv_fin(m):
                last = 4 * m + 3
                for p in range(2):
                    pv = [psPV.tile([65, 512], fp32, tag="pvT", name="pvT") for _ in range(2)]
                    for kk in range(4 * m + 4):
                        lo = max(0, kk - 4 * m) * 128
                        at = at_tiles[(m, kk)][p]
                        for hh in range(2):
                            h = 2 * p + hh
                            nc.tensor.matmul(
                                pv[hh][:, lo:512],
                                v_sb[h][:, 128 * kk : 128 * kk + 65],
                                at[:, 512 * hh + lo : 512 * hh + 512],
                                start=(kk == 0), stop=(kk == last))
                    for hh in range(2):
                        finalize(pv[hh], 2 * p + hh, m)

            # ---------------- output projection ------------------------
            def outproj(sc):
                scsl = slice(512 * sc, 512 * (sc + 1))
                for fo in range(F_CH):
                    fsl = slice(128 * fo, 128 * (fo + 1))
                    t = psProj.tile([128, 512], fp32, tag="proj", name="ow")
                    nc.tensor.matmul(t, wo[0][:, fsl], aoT[0][:, scsl], start=True, stop=False)
                    nc.tensor.matmul(t, wo[1][:, fsl], aoT[1][:, scsl], start=False, stop=True)
                    ow = fin.tile([128, 512], bf16, tag="ow", name="owd")
                    if fo % 2 == 0:
                        nc.vector.tensor_copy(ow, t)
                    else:
                        nc.scalar.copy(ow, t)
                    nc.sync.dma_start(out=outT_d[fsl, scsl], in_=ow)

            # ---------------- emission schedule ------------------------
            # exp-feeding chain first (ACT is the scarce engine); PV /
            # finalize / out-projection trail as PE filler.
            qk01_streamed()
            scores_exp(0)
            scores_exp(1)
            qk_proj(wq, wqb, q1, q2, 1)
            qk_proj(wk, wkb, k1, k2, 1)
            v_proj(0)
            v_transpose(0)
            v_proj(1)
            v_transpose(1)
            scores_exp(2)
            pv_fin(0)
            pv_fin(1)
            scores_exp(3)
            pv_fin(2)
            outproj(0)
            outproj(1)
            outproj(2)
            pv_fin(3)
            outproj(3)

    nc.compile()
    return nc


def _get_nc(with_bias):
    key = ("nc", with_bias)
    if key not in _CACHE:
        _CACHE[key] = _build_nc(with_bias)
    return _CACHE[key]


def _host_prep(x, positions, Wq, bq, Wk, bk, Wv, bv, Wo, bo):
    """Build the 8 per-core input maps."""
    ts = MAX_WAVELENGTH ** (2.0 * np.arange(HALF, dtype=np.float32) / D)  # [32]
    in_maps = []
    for c in range(NCORES):
        b, g = c // 4, c % 4
        heads = np.arange(4 * g, 4 * g + 4)
        cols_x1 = np.concatenate([64 * h + np.arange(32) for h in heads])
        cols_x2 = cols_x1 + 32
        perm = np.concatenate([cols_x1, cols_x2])
        vcols = np.concatenate([64 * h + np.arange(64) for h in heads])

        xT = np.empty((F + 1, S), dtype=BF16)
        xT[:F] = x[b].T.astype(BF16)
        xT[F] = 1.0

        wq_e = np.empty((F + 1, 256), dtype=BF16)
        wq_e[:F] = Wq[:, perm].astype(BF16)
        wq_e[F] = bq[perm].astype(BF16)
        wk_e = np.empty((F + 1, 256), dtype=BF16)
        wk_e[:F] = Wk[:, perm].astype(BF16)
        wk_e[F] = bk[perm].astype(BF16)
        wv_e = np.empty((F + 1, 256), dtype=BF16)
        wv_e[:F] = Wv[:, vcols].astype(BF16)
        wv_e[F] = bv[vcols].astype(BF16)

        pos = positions[b].astype(np.float32)  # [S]
        ang = pos[None, :] / ts[:, None]  # [32, S]
        cosw = np.cos(ang).astype(BF16)
        sinw = np.sin(ang).astype(BF16)

        ii = np.arange(128)
        mask = np.tile((ii[:, None] <= ii[None, :]).astype(BF16), (1, 2))

        in_maps.append({
            "xT": xT,
            "wq": wq_e,
            "wk": wk_e,
            "wv": wv_e,
            "wo": Wo[64 * heads[0] : 64 * heads[0] + 256, :].astype(BF16),
            "cosw": cosw,
            "sinw": sinw,
            "mask": np.ascontiguousarray(mask),
        })
    return in_maps


def kernel(x, positions, Wq, bq, Wk, bk, Wv, bv, Wo, bo):
    global LAST_RESULT
    from concourse.bass_utils import run_bass_kernel_spmd

    x = np.asarray(x, dtype=np.float32)
    positions = np.asarray(positions)
    args = [np.asarray(a, dtype=np.float32) for a in (Wq, bq, Wk, bk, Wv, bv, Wo, bo)]
    Wq, bq, Wk, bk, Wv, bv, Wo, bo = args

    with_bias = bool(np.any(bq) or np.any(bk) or np.any(bv))
    nc = _get_nc(with_bias)
    in_maps = _host_prep(x, positions, Wq, bq, Wk, bk, Wv, bv, Wo, bo)
    try:
        res = run_bass_kernel_spmd(nc, in_maps, core_ids=list(range(NCORES)))
    except ModuleNotFoundError:
        # axon NTFF profiling hook unavailable in this image; run untraced
        os.environ["BASS_NEVER_TRACE"] = "1"
        res = run_bass_kernel_spmd(nc, in_maps, core_ids=list(range(NCORES)))
    LAST_RESULT = res

    out = np.empty((B, S, F), dtype=np.float32)
    for b in range(B):
        acc = np.zeros((F, S), dtype=np.float32)
        for g in range(4):
            acc += res.results[4 * b + g]["outT"].astype(np.float32)
        out[b] = acc.T + bo[None, :]
    return out


# revision 14
# speedup vs baseline: 1.0572x; 1.0572x over previous
"""Multi-head causal attention with RoPE on 8 trn2 cores — v2.

Sharding: core c -> batch b = c // 4, head group g = c % 4 (heads 4g..4g+4).
Each core computes q/k/v projections for its 4 heads, causal attention, and
a partial output-projection (its heads' slice of Wo). The host sums the 4
bf16 partials per batch and adds the output bias.

v2 changes vs v1 (273us baseline):
  - All projections are weight-stationary with n-pair grouping (each
    LDWEIGHTS feeds 2 matmuls); v is projected transposed like q/k and
    moved to [s, d] PV layout with DMA-xbar transposes off the PE.
  - Biases fold into the matmuls via a bias row x ones row; when all
    biases are zero (the bench case) that path compiles out entirely.
  - Emission is software-pipelined for the scarce engines: the exp-feeding
    chain (projections n01 -> scores/exp m0/m1 -> projections n23 ->
    scores/exp m2/m3) is emitted ahead of the PV/finalize/out-projection
    chain, which fills PE gaps; keeps ACT (exp ~64us floor) saturated and
    the PE dense enough that HAM stays at K=8/8 (v1 ran attention at
    half-clock for 153us).
  - Input DMAs ordered so the first matmul starts ~2us in; output is bf16,
    written per [128,512] chunk as the out-projection drains.
"""

import os

import numpy as np
import ml_dtypes

BF16 = ml_dtypes.bfloat16

B, S, F = 2, 2048, 1024
H, D = 16, 64
HALF = D // 2
NCORES = 8
HPC = 4  # heads per core
S_TILES = S // 128  # 16
N_CH = S // 512  # 4
F_CH = F // 128  # 8
MACROS = 4
MAX_WAVELENGTH = 10000.0

_CACHE = {}
LAST_RESULT = None


def _build_nc(with_bias):
    import concourse.bacc as bacc
    import concourse.tile as tile
    import concourse.mybir as mybir

    fp32 = mybir.dt.float32
    bf16 = mybir.dt.bfloat16
    MULT = mybir.AluOpType.mult
    EXP = mybir.ActivationFunctionType.Exp

    nc = bacc.Bacc("TRN2", target_bir_lowering=False, debug=False)

    xT_d = nc.dram_tensor("xT", [F + 1, S], bf16, kind="ExternalInput")
    wq_d = nc.dram_tensor("wq", [F + 1, 256], bf16, kind="ExternalInput")
    wk_d = nc.dram_tensor("wk", [F + 1, 256], bf16, kind="ExternalInput")
    wv_d = nc.dram_tensor("wv", [F + 1, 256], bf16, kind="ExternalInput")
    wo_d = nc.dram_tensor("wo", [256, F], bf16, kind="ExternalInput")
    cos_d = nc.dram_tensor("cosw", [128, S], bf16, kind="ExternalInput")
    sin_d = nc.dram_tensor("sinw", [128, S], bf16, kind="ExternalInput")
    mask_d = nc.dram_tensor("mask", [128, 256], bf16, kind="ExternalInput")
    outT_d = nc.dram_tensor("outT", [F, S], bf16, kind="ExternalOutput")

    with tile.TileContext(nc) as tc:
        with (
            tc.tile_pool(name="persist", bufs=1) as persist,
            tc.tile_pool(name="tmp", bufs=12) as tmp,
            tc.tile_pool(name="attn", bufs=29 if not with_bias else 25) as attn_pool,
            tc.tile_pool(name="fin", bufs=3) as fin,
            tc.tile_pool(name="psProj", bufs=2, space="PSUM") as psProj,
            tc.tile_pool(name="psSps", bufs=2, space="PSUM") as psSps,
            tc.tile_pool(name="psPV", bufs=2, space="PSUM") as psPV,
        ):
            # ---------------- persistent tiles + ordered input DMAs -----
            # order = first-use order: wq+xT gate the first matmul, wk the
            # second projection, cos/sin the RoPE, wv/wo come later
            cosw = persist.tile([128, S], bf16, tag="cosw", name="cosw")
            sinw = persist.tile([128, S], bf16, tag="sinw", name="sinw")
            maskt = persist.tile([128, 256], bf16, tag="maskt", name="maskt")
            wq = [persist.tile([128, 256], bf16, tag=f"wq{i}", name=f"wq{i}") for i in range(F_CH)]
            xT = [persist.tile([128, S], bf16, tag=f"xT{i}", name=f"xT{i}") for i in range(F_CH)]
            wk = [persist.tile([128, 256], bf16, tag=f"wk{i}", name=f"wk{i}") for i in range(F_CH)]
            wv = [persist.tile([128, 256], bf16, tag=f"wv{i}", name=f"wv{i}") for i in range(F_CH)]
            wo = [persist.tile([128, F], bf16, tag=f"wo{i}", name=f"wo{i}") for i in range(2)]
            for i in range(F_CH):
                nc.sync.dma_start(out=wq[i], in_=wq_d[128 * i : 128 * (i + 1), :])
            for i in range(F_CH):
                nc.sync.dma_start(out=xT[i], in_=xT_d[128 * i : 128 * (i + 1), :])
            for i in range(F_CH):
                nc.sync.dma_start(out=wk[i], in_=wk_d[128 * i : 128 * (i + 1), :])
            nc.sync.dma_start(out=cosw, in_=cos_d[:, :])
            nc.sync.dma_start(out=sinw, in_=sin_d[:, :])
            nc.sync.dma_start(out=maskt, in_=mask_d[:, :])
            for i in range(F_CH):
                nc.sync.dma_start(out=wv[i], in_=wv_d[128 * i : 128 * (i + 1), :])
            for i in range(2):
                nc.sync.dma_start(out=wo[i], in_=wo_d[128 * i : 128 * (i + 1), :])

            if with_bias:
                xones = persist.tile([1, S], bf16, tag="xones", name="xones")
                wqb = persist.tile([1, 256], bf16, tag="wqb", name="wqb")
                wkb = persist.tile([1, 256], bf16, tag="wkb", name="wkb")
                wvb = persist.tile([1, 256], bf16, tag="wvb", name="wvb")
                nc.sync.dma_start(out=xones, in_=xT_d[F : F + 1, :])
                nc.sync.dma_start(out=wqb, in_=wq_d[F : F + 1, :])
                nc.sync.dma_start(out=wkb, in_=wk_d[F : F + 1, :])
                nc.sync.dma_start(out=wvb, in_=wv_d[F : F + 1, :])
            else:
                xones = wqb = wkb = wvb = None

            # PE warm-up: ~20 no-dep matmuls flip the HAM clock gate to
            # K=8/8 (~3.4us of activity) before the real data arrives, so
            # the DMA-paced first projection runs at 2.4 GHz
            wdum = persist.tile([128, 128], bf16, tag="wdum", name="wdum")
            xdum = persist.tile([128, 512], bf16, tag="xdum", name="xdum")
            nc.vector.memset(wdum[:, :], 0.0)
            nc.vector.memset(xdum[:, :], 0.0)
            tdum = psProj.tile([128, 512], fp32, tag="proj", name="tdum")
            for _ in range(20):
                nc.tensor.matmul(tdum, wdum, xdum, start=True, stop=True)

            # post-RoPE q/k, [d, s]; chunk 1 = x1 halves of the 4 heads
            # (head h -> partitions 32h..32h+32), chunk 2 = x2 halves.
            q1 = persist.tile([128, S], bf16, tag="q1", name="q1")
            q2 = persist.tile([128, S], bf16, tag="q2", name="q2")
            k1 = persist.tile([128, S], bf16, tag="k1", name="k1")
            k2 = persist.tile([128, S], bf16, tag="k2", name="k2")
            # v projected transposed ([d, s], head-major rows), then moved
            # to [s, d] layout; head h cols 65h..65h+64, col 65h+64 = ones.
            vT = [persist.tile([128, S], bf16, tag=f"vT{i}", name=f"vT{i}") for i in range(2)]
            # v in [s, d] layout: per-head tile, s-chunk st at cols
            # 128*st (64 dims + denominator-ones col at 128*st+64); the
            # 128-col stride keeps every xbar-transpose run 256B-aligned
            v_sb = [persist.tile([128, S], bf16, tag=f"vs{h}", name=f"vs{h}")
                    for h in range(HPC)]
            # attention output, [dh, s] (head h -> tile h//2 rows 64*(h%2))
            aoT = [persist.tile([128, S], bf16, tag=f"aoT{i}", name=f"aoT{i}") for i in range(2)]

            # ones columns of v_sb (denominator accumulators); the v
            # transposes write disjoint columns so order doesn't matter
            for hl in range(HPC):
                ones_v = v_sb[hl][:, :].rearrange("p (t c) -> p t c", c=128)[:, :, 64:65]
                nc.gpsimd.memset(ones_v, 1.0)

            # ---------------- weight-stationary projections -------------
            def qk01_streamed():
                """q n0/n1 (both halves) + k n0/n1, paced by xT chunk
                arrival: q uses the attention-phase psSps banks (idle here),
                k halfA rides the same kc sweep through psProj, halfB
                follows once halfA has drained."""
                qA = psSps.tile([128, 1024], fp32, tag="sps", name="qA")
                qB = psSps.tile([128, 1024], fp32, tag="sps", name="qB")
                kA = [psProj.tile([128, 512], fp32, tag="proj", name=f"kA{j}") for j in range(2)]
                for kc in range(F_CH):
                    st = (kc == 0)
                    sp = (kc == F_CH - 1 and not with_bias)
                    for j in range(2):
                        nc.tensor.matmul(qA[:, 512 * j : 512 * (j + 1)], wq[kc][:, 0:128],
                                         xT[kc][:, 512 * j : 512 * (j + 1)], start=st, stop=sp)
                        nc.tensor.matmul(qB[:, 512 * j : 512 * (j + 1)], wq[kc][:, 128:256],
                                         xT[kc][:, 512 * j : 512 * (j + 1)], start=st, stop=sp)
                        nc.tensor.matmul(kA[j], wk[kc][:, 0:128],
                                         xT[kc][:, 512 * j : 512 * (j + 1)], start=st, stop=sp)
                if with_bias:
                    for j in range(2):
                        jsl = slice(512 * j, 512 * (j + 1))
                        nc.tensor.matmul(qA[:, jsl], wqb[0:1, 0:128], xones[:, jsl],
                                         start=False, stop=True)
                        nc.tensor.matmul(qB[:, jsl], wqb[0:1, 128:256], xones[:, jsl],
                                         start=False, stop=True)
                        nc.tensor.matmul(kA[j], wkb[0:1, 0:128], xones[:, jsl],
                                         start=False, stop=True)
                cq = [[None, None], [None, None]]
                ck = [[None, None], [None, None]]
                for j in range(2):
                    cq[0][j] = tmp.tile([128, 512], bf16, tag="rope", name="cq0")
                    nc.scalar.copy(cq[0][j], qA[:, 512 * j : 512 * (j + 1)])
                    cq[1][j] = tmp.tile([128, 512], bf16, tag="rope", name="cq1")
                    nc.scalar.copy(cq[1][j], qB[:, 512 * j : 512 * (j + 1)])
                    ck[0][j] = tmp.tile([128, 512], bf16, tag="rope", name="ck0")
                    nc.scalar.copy(ck[0][j], kA[j])
                kB = [psProj.tile([128, 512], fp32, tag="proj", name=f"kB{j}") for j in range(2)]
                for kc in range(F_CH):
                    for j in range(2):
                        nc.tensor.matmul(kB[j], wk[kc][:, 128:256],
                                         xT[kc][:, 512 * j : 512 * (j + 1)],
                                         start=(kc == 0),
                                         stop=(kc == F_CH - 1 and not with_bias))
                if with_bias:
                    for j in range(2):
                        nc.tensor.matmul(kB[j], wkb[0:1, 128:256],
                                         xones[:, 512 * j : 512 * (j + 1)],
                                         start=False, stop=True)
                for j in range(2):
                    ck[1][j] = tmp.tile([128, 512], bf16, tag="rope", name="ck1")
                    nc.scalar.copy(ck[1][j], kB[j])
                for cs, o1, o2 in ((cq, q1, q2), (ck, k1, k2)):
                    for j in range(2):
                        nsl = slice(512 * j, 512 * (j + 1))
                        c1, c2 = cs[0][j], cs[1][j]
                        t1 = tmp.tile([128, 512], bf16, tag="rope", name="t1")
                        t2 = tmp.tile([128, 512], bf16, tag="rope", name="t2")
                        t3 = tmp.tile([128, 512], bf16, tag="rope", name="t3")
                        t4 = tmp.tile([128, 512], bf16, tag="rope", name="t4")
                        nc.vector.tensor_mul(t1, c1, cosw[:, nsl])
                        nc.vector.tensor_mul(t2, c2, sinw[:, nsl])
                        nc.vector.tensor_mul(t3, c2, cosw[:, nsl])
                        nc.vector.tensor_mul(t4, c1, sinw[:, nsl])
                        nc.vector.tensor_sub(o1[:, nsl], t1, t2)
                        nc.vector.tensor_add(o2[:, nsl], t3, t4)

            def wproj(w_sb, wb, npair, drains):
                """One projection, s-columns [1024*npair, 1024*(npair+1)).
                drains(half, j, ps) consumes the psum tile for n=2*npair+j."""
                for half in range(2):
                    hsl = slice(128 * half, 128 * (half + 1))
                    ts = [psProj.tile([128, 512], fp32, tag="proj", name=f"pj{j}") for j in range(2)]
                    for kc in range(F_CH):
                        for j in range(2):
                            n = 2 * npair + j
                            nc.tensor.matmul(ts[j], w_sb[kc][:, hsl],
                                             xT[kc][:, 512 * n : 512 * (n + 1)],
                                             start=(kc == 0),
                                             stop=(kc == F_CH - 1 and not with_bias))
                    if with_bias:
                        for j in range(2):
                            n = 2 * npair + j
                            nc.tensor.matmul(ts[j], wb[0:1, hsl],
                                             xones[:, 512 * n : 512 * (n + 1)],
                                             start=False, stop=True)
                    for j in range(2):
                        drains(half, j, ts[j])

            def qk_proj(w_sb, wb, o1, o2, npair):
                cs = [[None, None], [None, None]]

                def drains(half, j, ps):
                    c = tmp.tile([128, 512], bf16, tag="rope", name="c")
                    nc.scalar.copy(c, ps)
                    cs[half][j] = c

                wproj(w_sb, wb, npair, drains)
                for j in range(2):
                    n = 2 * npair + j
                    nsl = slice(512 * n, 512 * (n + 1))
                    c1, c2 = cs[0][j], cs[1][j]
                    t1 = tmp.tile([128, 512], bf16, tag="rope", name="t1")
                    t2 = tmp.tile([128, 512], bf16, tag="rope", name="t2")
                    t3 = tmp.tile([128, 512], bf16, tag="rope", name="t3")
                    t4 = tmp.tile([128, 512], bf16, tag="rope", name="t4")
                    nc.vector.tensor_mul(t1, c1, cosw[:, nsl])
                    nc.vector.tensor_mul(t2, c2, sinw[:, nsl])
                    nc.vector.tensor_mul(t3, c2, cosw[:, nsl])
                    nc.vector.tensor_mul(t4, c1, sinw[:, nsl])
                    nc.vector.tensor_sub(o1[:, nsl], t1, t2)
                    nc.vector.tensor_add(o2[:, nsl], t3, t4)

            def v_proj(npair):
                def drains(half, j, ps):
                    n = 2 * npair + j
                    nc.vector.tensor_copy(vT[half][:, 512 * n : 512 * (n + 1)], ps)

                wproj(wv, wvb, npair, drains)

            def v_transpose():
                # one strided xbar transpose per head: [64, 2048] ->
                # [(128x16), 64] with the 16 s-chunks 128 cols apart
                for half in range(2):
                    for hh in range(2):
                        hl = 2 * half + hh
                        out_v = v_sb[hl][:, :].rearrange("p (t c) -> p t c", c=128)[:, :, 0:64]
                        nc.sync.dma_start_transpose(
                            out=out_v, in_=vT[half][64 * hh : 64 * (hh + 1), :])

            # ---------------- attention -------------------------------
            at_tiles = {}  # (m, kk) -> [at_p0, at_p1]

            def scores_exp(m):
                for kk in range(4 * m + 4):
                    t = kk - 4 * m
                    lo = max(0, t) * 128
                    ksl = slice(128 * kk, 128 * (kk + 1))
                    qsl = slice(512 * m + lo, 512 * (m + 1))
                    sps_l = [psSps.tile([128, 1024], fp32, tag="sps", name=f"sps{p}")
                             for p in range(2)]
                    # wave order: all 4 heads' k1 matmuls (4 concurrent
                    # row-groups), then the k2 accumulate wave
                    for qq, kx in ((q1, k1), (q2, k2)):
                        for p in range(2):
                            for hh in range(2):
                                h = 2 * p + hh
                                hp = slice(32 * h, 32 * (h + 1))
                                osl = slice(512 * hh + lo, 512 * hh + 512)
                                nc.tensor.matmul(sps_l[p][:, osl], kx[hp, ksl], qq[hp, qsl],
                                                 start=(kx is k1), stop=(kx is k2),
                                                 tile_position=(32 * h, 0))
                    atp = []
                    for p in range(2):
                        at = attn_pool.tile([128, 1024], bf16, tag="attn", name="at")
                        sps_v = sps_l[p][:, :].rearrange("a (h q) -> a h q", h=2)[:, :, lo:512]
                        at_v = at[:, :].rearrange("a (h q) -> a h q", h=2)[:, :, lo:512]
                        nc.scalar.activation(out=at_v, in_=sps_v, func=EXP, scale=0.125)
                        if t >= 0:
                            dv = at[:, :].rearrange("a (h q) -> a h q", h=2)[:, :, 128 * t : 128 * (t + 1)]
                            nc.vector.tensor_tensor(dv, dv, maskt, op=MULT)
                        atp.append(at)
                    at_tiles[(m, kk)] = atp

            def finalize(pvt, h, m):
                msl = slice(512 * m, 512 * (m + 1))
                cix, r0 = h // 2, 64 * (h % 2)
                s65 = fin.tile([65, 512], fp32, tag="s65", name="s65")
                nc.vector.tensor_copy(s65, pvt[0:65, :])
                rec0 = fin.tile([1, 512], fp32, tag="rec0", name="rec0")
                nc.sync.dma_start(out=rec0, in_=s65[64:65, :])
                rcp = fin.tile([1, 512], fp32, tag="rcp", name="rcp")
                nc.vector.reciprocal_approx_fast(rcp, rec0)
                rb = fin.tile([64, 512], fp32, tag="rb", name="rb")
                nc.gpsimd.partition_broadcast(rb, rcp[0:1, :])
                if r0 == 0:
                    nc.vector.tensor_tensor(aoT[cix][0:64, msl], s65[0:64, :], rb, op=MULT)
                else:
                    ao = fin.tile([64, 512], bf16, tag="ao", name="ao")
                    nc.vector.tensor_tensor(ao, s65[0:64, :], rb, op=MULT)
                    nc.sync.dma_start(out=aoT[cix][64:128, msl], in_=ao)

            def pv_fin(m):
                last = 4 * m + 3
                for p in range(2):
                    pv = [psPV.tile([65, 512], fp32, tag="pvT", name="pvT") for _ in range(2)]
                    for kk in range(4 * m + 4):
                        lo = max(0, kk - 4 * m) * 128
                        at = at_tiles[(m, kk)][p]
                        for hh in range(2):
                            h = 2 * p + hh
                            nc.tensor.matmul(
                                pv[hh][:, lo:512],
                                v_sb[h][:, 128 * kk : 128 * kk + 65],
                                at[:, 512 * hh + lo : 512 * hh + 512],
                                start=(kk == 0), stop=(kk == last))
                    for hh in range(2):
                        finalize(pv[hh], 2 * p + hh, m)

            # ---------------- output projection ------------------------
            def outproj(sc):
                scsl = slice(512 * sc, 512 * (sc + 1))
                for fo in range(F_CH):
                    fsl = slice(128 * fo, 128 * (fo + 1))
                    t = psProj.tile([128, 512], fp32, tag="proj", name="ow")
                    nc.tensor.matmul(t, wo[0][:, fsl], aoT[0][:, scsl], start=True, stop=False)
                    nc.tensor.matmul(t, wo[1][:, fsl], aoT[1][:, scsl], start=False, stop=True)
                    ow = fin.tile([128, 512], bf16, tag="ow", name="owd")
                    if fo % 2 == 0:
                        nc.vector.tensor_copy(ow, t)
                    else:
                        nc.scalar.copy(ow, t)
                    nc.sync.dma_start(out=outT_d[fsl, scsl], in_=ow)

            # ---------------- emission schedule ------------------------
            # exp-feeding chain first (ACT is the scarce engine); PV /
            # finalize / out-projection trail as PE filler.
            qk01_streamed()
            scores_exp(0)
            scores_exp(1)
            qk_proj(wq, wqb, q1, q2, 1)
            qk_proj(wk, wkb, k1, k2, 1)
            v_proj(0)
            v_proj(1)
            v_transpose()
            scores_exp(2)
            pv_fin(0)
            pv_fin(1)
            scores_exp(3)
            pv_fin(2)
            outproj(0)
            outproj(1)
            pv_fin(3)
            outproj(2)
            outproj(3)

    nc.compile()
    return nc


def _get_nc(with_bias):
    key = ("nc", with_bias)
    if key not in _CACHE:
        _CACHE[key] = _build_nc(with_bias)
    return _CACHE[key]


def _host_prep(x, positions, Wq, bq, Wk, bk, Wv, bv, Wo, bo):
    """Build the 8 per-core input maps."""
    ts = MAX_WAVELENGTH ** (2.0 * np.arange(HALF, dtype=np.float32) / D)  # [32]
    in_maps = []
    for c in range(NCORES):
        b, g = c // 4, c % 4
        heads = np.arange(4 * g, 4 * g + 4)
        cols_x1 = np.concatenate([64 * h + np.arange(32) for h in heads])
        cols_x2 = cols_x1 + 32
        perm = np.concatenate([cols_x1, cols_x2])
        vcols = np.concatenate([64 * h + np.arange(64) for h in heads])

        xT = np.empty((F + 1, S), dtype=BF16)
        xT[:F] = x[b].T.astype(BF16)
        xT[F] = 1.0

        wq_e = np.empty((F + 1, 256), dtype=BF16)
        wq_e[:F] = Wq[:, perm].astype(BF16)
        wq_e[F] = bq[perm].astype(BF16)
        wk_e = np.empty((F + 1, 256), dtype=BF16)
        wk_e[:F] = Wk[:, perm].astype(BF16)
        wk_e[F] = bk[perm].astype(BF16)
        wv_e = np.empty((F + 1, 256), dtype=BF16)
        wv_e[:F] = Wv[:, vcols].astype(BF16)
        wv_e[F] = bv[vcols].astype(BF16)

        pos = positions[b].astype(np.float32)  # [S]
        ang = pos[None, :] / ts[:, None]  # [32, S]
        cosw = np.tile(np.cos(ang), (4, 1)).astype(BF16)
        sinw = np.tile(np.sin(ang), (4, 1)).astype(BF16)

        ii = np.arange(128)
        mask = np.tile((ii[:, None] <= ii[None, :]).astype(BF16), (1, 2))

        in_maps.append({
            "xT": xT,
            "wq": wq_e,
            "wk": wk_e,
            "wv": wv_e,
            "wo": Wo[64 * heads[0] : 64 * heads[0] + 256, :].astype(BF16),
            "cosw": cosw,
            "sinw": sinw,
            "mask": np.ascontiguousarray(mask),
        })
    return in_maps


def kernel(x, positions, Wq, bq, Wk, bk, Wv, bv, Wo, bo):
    global LAST_RESULT
    from concourse.bass_utils import run_bass_kernel_spmd

    x = np.asarray(x, dtype=np.float32)
    positions = np.asarray(positions)
    args = [np.asarray(a, dtype=np.float32) for a in (Wq, bq, Wk, bk, Wv, bv, Wo, bo)]
    Wq, bq, Wk, bk, Wv, bv, Wo, bo = args

    with_bias = bool(np.any(bq) or np.any(bk) or np.any(bv))
    nc = _get_nc(with_bias)
    in_maps = _host_prep(x, positions, Wq, bq, Wk, bk, Wv, bv, Wo, bo)
    try:
        res = run_bass_kernel_spmd(nc, in_maps, core_ids=list(range(NCORES)))
    except ModuleNotFoundError:
        # axon NTFF profiling hook unavailable in this image; run untraced
        os.environ["BASS_NEVER_TRACE"] = "1"
        res = run_bass_kernel_spmd(nc, in_maps, core_ids=list(range(NCORES)))
    LAST_RESULT = res

    out = np.empty((B, S, F), dtype=np.float32)
    for b in range(B):
        acc = np.zeros((F, S), dtype=np.float32)
        for g in range(4):
            acc += res.results[4 * b + g]["outT"].astype(np.float32)
        out[b] = acc.T + bo[None, :]
    return out


# revision 16
# speedup vs baseline: 1.0725x; 1.0144x over previous
"""Multi-head causal attention with RoPE on 8 trn2 cores — v2.

Sharding: core c -> batch b = c // 4, head group g = c % 4 (heads 4g..4g+4).
Each core computes q/k/v projections for its 4 heads, causal attention, and
a partial output-projection (its heads' slice of Wo). The host sums the 4
bf16 partials per batch and adds the output bias.

v2 changes vs v1 (273us baseline):
  - All projections are weight-stationary with n-pair grouping (each
    LDWEIGHTS feeds 2 matmuls); v is projected transposed like q/k and
    moved to [s, d] PV layout with DMA-xbar transposes off the PE.
  - Biases fold into the matmuls via a bias row x ones row; when all
    biases are zero (the bench case) that path compiles out entirely.
  - Emission is software-pipelined for the scarce engines: the exp-feeding
    chain (projections n01 -> scores/exp m0/m1 -> projections n23 ->
    scores/exp m2/m3) is emitted ahead of the PV/finalize/out-projection
    chain, which fills PE gaps; keeps ACT (exp ~64us floor) saturated and
    the PE dense enough that HAM stays at K=8/8 (v1 ran attention at
    half-clock for 153us).
  - Input DMAs ordered so the first matmul starts ~2us in; output is bf16,
    written per [128,512] chunk as the out-projection drains.
"""

import os

import numpy as np
import ml_dtypes

BF16 = ml_dtypes.bfloat16

B, S, F = 2, 2048, 1024
H, D = 16, 64
HALF = D // 2
NCORES = 8
HPC = 4  # heads per core
S_TILES = S // 128  # 16
N_CH = S // 512  # 4
F_CH = F // 128  # 8
MACROS = 4
MAX_WAVELENGTH = 10000.0

_CACHE = {}
LAST_RESULT = None


def _build_nc(with_bias):
    import concourse.bacc as bacc
    import concourse.tile as tile
    import concourse.mybir as mybir

    fp32 = mybir.dt.float32
    bf16 = mybir.dt.bfloat16
    MULT = mybir.AluOpType.mult
    EXP = mybir.ActivationFunctionType.Exp

    nc = bacc.Bacc("TRN2", target_bir_lowering=False, debug=False)

    xT_d = nc.dram_tensor("xT", [F + 1, S], bf16, kind="ExternalInput")
    wq_d = nc.dram_tensor("wq", [F + 1, 256], bf16, kind="ExternalInput")
    wk_d = nc.dram_tensor("wk", [F + 1, 256], bf16, kind="ExternalInput")
    wv_d = nc.dram_tensor("wv", [F + 1, 256], bf16, kind="ExternalInput")
    wo_d = nc.dram_tensor("wo", [256, F], bf16, kind="ExternalInput")
    cos_d = nc.dram_tensor("cosw", [128, S], bf16, kind="ExternalInput")
    sin_d = nc.dram_tensor("sinw", [128, S], bf16, kind="ExternalInput")
    mask_d = nc.dram_tensor("mask", [128, 256], bf16, kind="ExternalInput")
    outT_d = nc.dram_tensor("outT", [F, S], bf16, kind="ExternalOutput")

    with tile.TileContext(nc) as tc:
        with (
            tc.tile_pool(name="persist", bufs=1) as persist,
            tc.tile_pool(name="tmp", bufs=12) as tmp,
            tc.tile_pool(name="attn", bufs=29 if not with_bias else 25) as attn_pool,
            tc.tile_pool(name="fin", bufs=3) as fin,
            tc.tile_pool(name="psProj", bufs=2, space="PSUM") as psProj,
            tc.tile_pool(name="psSps", bufs=2, space="PSUM") as psSps,
            tc.tile_pool(name="psPV", bufs=2, space="PSUM") as psPV,
        ):
            # ---------------- persistent tiles + ordered input DMAs -----
            # order = first-use order: wq+xT gate the first matmul, wk the
            # second projection, cos/sin the RoPE, wv/wo come later
            cosw = persist.tile([128, S], bf16, tag="cosw", name="cosw")
            sinw = persist.tile([128, S], bf16, tag="sinw", name="sinw")
            maskt = persist.tile([128, 256], bf16, tag="maskt", name="maskt")
            wq = [persist.tile([128, 256], bf16, tag=f"wq{i}", name=f"wq{i}") for i in range(F_CH)]
            xT = [persist.tile([128, S], bf16, tag=f"xT{i}", name=f"xT{i}") for i in range(F_CH)]
            wk = [persist.tile([128, 256], bf16, tag=f"wk{i}", name=f"wk{i}") for i in range(F_CH)]
            wv = [persist.tile([128, 256], bf16, tag=f"wv{i}", name=f"wv{i}") for i in range(F_CH)]
            wo = [persist.tile([128, F], bf16, tag=f"wo{i}", name=f"wo{i}") for i in range(2)]
            for i in range(F_CH):
                nc.sync.dma_start(out=wq[i], in_=wq_d[128 * i : 128 * (i + 1), :])
            for i in range(F_CH):
                nc.sync.dma_start(out=xT[i], in_=xT_d[128 * i : 128 * (i + 1), :])
            for i in range(F_CH):
                nc.sync.dma_start(out=wk[i], in_=wk_d[128 * i : 128 * (i + 1), :])
            nc.sync.dma_start(out=cosw, in_=cos_d[:, :])
            nc.sync.dma_start(out=sinw, in_=sin_d[:, :])
            nc.sync.dma_start(out=maskt, in_=mask_d[:, :])
            for i in range(F_CH):
                nc.sync.dma_start(out=wv[i], in_=wv_d[128 * i : 128 * (i + 1), :])
            for i in range(2):
                nc.sync.dma_start(out=wo[i], in_=wo_d[128 * i : 128 * (i + 1), :])

            if with_bias:
                xones = persist.tile([1, S], bf16, tag="xones", name="xones")
                wqb = persist.tile([1, 256], bf16, tag="wqb", name="wqb")
                wkb = persist.tile([1, 256], bf16, tag="wkb", name="wkb")
                wvb = persist.tile([1, 256], bf16, tag="wvb", name="wvb")
                nc.sync.dma_start(out=xones, in_=xT_d[F : F + 1, :])
                nc.sync.dma_start(out=wqb, in_=wq_d[F : F + 1, :])
                nc.sync.dma_start(out=wkb, in_=wk_d[F : F + 1, :])
                nc.sync.dma_start(out=wvb, in_=wv_d[F : F + 1, :])
            else:
                xones = wqb = wkb = wvb = None

            # PE warm-up: ~20 no-dep matmuls flip the HAM clock gate to
            # K=8/8 (~3.4us of activity) before the real data arrives, so
            # the DMA-paced first projection runs at 2.4 GHz
            wdum = persist.tile([128, 128], bf16, tag="wdum", name="wdum")
            xdum = persist.tile([128, 512], bf16, tag="xdum", name="xdum")
            nc.vector.memset(wdum[:, :], 0.0)
            nc.vector.memset(xdum[:, :], 0.0)
            tdum = psProj.tile([128, 512], fp32, tag="proj", name="tdum")
            for _ in range(20):
                nc.tensor.matmul(tdum, wdum, xdum, start=True, stop=True)

            # post-RoPE q/k, [d, s]; chunk 1 = x1 halves of the 4 heads
            # (head h -> partitions 32h..32h+32), chunk 2 = x2 halves.
            q1 = persist.tile([128, S], bf16, tag="q1", name="q1")
            q2 = persist.tile([128, S], bf16, tag="q2", name="q2")
            k1 = persist.tile([128, S], bf16, tag="k1", name="k1")
            k2 = persist.tile([128, S], bf16, tag="k2", name="k2")
            # v projected transposed ([d, s], head-major rows), then moved
            # to [s, d] layout; head h cols 65h..65h+64, col 65h+64 = ones.
            vT = [persist.tile([128, S], bf16, tag=f"vT{i}", name=f"vT{i}") for i in range(2)]
            # v in [s, d] layout: per-head tile, s-chunk st at cols
            # 128*st (64 dims + denominator-ones col at 128*st+64); the
            # 128-col stride keeps every xbar-transpose run 256B-aligned
            v_sb = [persist.tile([128, S], bf16, tag=f"vs{h}", name=f"vs{h}")
                    for h in range(HPC)]
            # attention output, [dh, s] (head h -> tile h//2 rows 64*(h%2))
            aoT = [persist.tile([128, S], bf16, tag=f"aoT{i}", name=f"aoT{i}") for i in range(2)]

            # ones columns of v_sb (denominator accumulators); the v
            # transposes write disjoint columns so order doesn't matter
            for hl in range(HPC):
                ones_v = v_sb[hl][:, :].rearrange("p (t c) -> p t c", c=128)[:, :, 64:65]
                nc.gpsimd.memset(ones_v, 1.0)

            # ---------------- weight-stationary projections -------------
            def qk01_streamed():
                """q n0/n1 (both halves) + k n0/n1, paced by xT chunk
                arrival: q uses the attention-phase psSps banks (idle here),
                k halfA rides the same kc sweep through psProj, halfB
                follows once halfA has drained."""
                qA = psSps.tile([128, 1024], fp32, tag="sps", name="qA")
                qB = psSps.tile([128, 1024], fp32, tag="sps", name="qB")
                kA = [psProj.tile([128, 512], fp32, tag="proj", name=f"kA{j}") for j in range(2)]
                for kc in range(F_CH):
                    st = (kc == 0)
                    sp = (kc == F_CH - 1 and not with_bias)
                    for j in range(2):
                        nc.tensor.matmul(qA[:, 512 * j : 512 * (j + 1)], wq[kc][:, 0:128],
                                         xT[kc][:, 512 * j : 512 * (j + 1)], start=st, stop=sp)
                        nc.tensor.matmul(qB[:, 512 * j : 512 * (j + 1)], wq[kc][:, 128:256],
                                         xT[kc][:, 512 * j : 512 * (j + 1)], start=st, stop=sp)
                        nc.tensor.matmul(kA[j], wk[kc][:, 0:128],
                                         xT[kc][:, 512 * j : 512 * (j + 1)], start=st, stop=sp)
                if with_bias:
                    for j in range(2):
                        jsl = slice(512 * j, 512 * (j + 1))
                        nc.tensor.matmul(qA[:, jsl], wqb[0:1, 0:128], xones[:, jsl],
                                         start=False, stop=True)
                        nc.tensor.matmul(qB[:, jsl], wqb[0:1, 128:256], xones[:, jsl],
                                         start=False, stop=True)
                        nc.tensor.matmul(kA[j], wkb[0:1, 0:128], xones[:, jsl],
                                         start=False, stop=True)
                cq = [[None, None], [None, None]]
                ck = [[None, None], [None, None]]
                for j in range(2):
                    cq[0][j] = tmp.tile([128, 512], bf16, tag="rope", name="cq0")
                    nc.scalar.copy(cq[0][j], qA[:, 512 * j : 512 * (j + 1)])
                    cq[1][j] = tmp.tile([128, 512], bf16, tag="rope", name="cq1")
                    nc.scalar.copy(cq[1][j], qB[:, 512 * j : 512 * (j + 1)])
                    ck[0][j] = tmp.tile([128, 512], bf16, tag="rope", name="ck0")
                    nc.scalar.copy(ck[0][j], kA[j])
                kB = [psProj.tile([128, 512], fp32, tag="proj", name=f"kB{j}") for j in range(2)]
                for kc in range(F_CH):
                    for j in range(2):
                        nc.tensor.matmul(kB[j], wk[kc][:, 128:256],
                                         xT[kc][:, 512 * j : 512 * (j + 1)],
                                         start=(kc == 0),
                                         stop=(kc == F_CH - 1 and not with_bias))
                if with_bias:
                    for j in range(2):
                        nc.tensor.matmul(kB[j], wkb[0:1, 128:256],
                                         xones[:, 512 * j : 512 * (j + 1)],
                                         start=False, stop=True)
                for j in range(2):
                    ck[1][j] = tmp.tile([128, 512], bf16, tag="rope", name="ck1")
                    nc.scalar.copy(ck[1][j], kB[j])
                for cs, o1, o2 in ((cq, q1, q2), (ck, k1, k2)):
                    for j in range(2):
                        nsl = slice(512 * j, 512 * (j + 1))
                        c1, c2 = cs[0][j], cs[1][j]
                        t1 = tmp.tile([128, 512], bf16, tag="rope", name="t1")
                        t2 = tmp.tile([128, 512], bf16, tag="rope", name="t2")
                        t3 = tmp.tile([128, 512], bf16, tag="rope", name="t3")
                        t4 = tmp.tile([128, 512], bf16, tag="rope", name="t4")
                        nc.vector.tensor_mul(t1, c1, cosw[:, nsl])
                        nc.vector.tensor_mul(t2, c2, sinw[:, nsl])
                        nc.vector.tensor_mul(t3, c2, cosw[:, nsl])
                        nc.vector.tensor_mul(t4, c1, sinw[:, nsl])
                        nc.vector.tensor_sub(o1[:, nsl], t1, t2)
                        nc.vector.tensor_add(o2[:, nsl], t3, t4)

            def wproj(w_sb, wb, npair, drains):
                """One projection, s-columns [1024*npair, 1024*(npair+1)).
                drains(half, j, ps) consumes the psum tile for n=2*npair+j."""
                for half in range(2):
                    hsl = slice(128 * half, 128 * (half + 1))
                    ts = [psProj.tile([128, 512], fp32, tag="proj", name=f"pj{j}") for j in range(2)]
                    for kc in range(F_CH):
                        for j in range(2):
                            n = 2 * npair + j
                            nc.tensor.matmul(ts[j], w_sb[kc][:, hsl],
                                             xT[kc][:, 512 * n : 512 * (n + 1)],
                                             start=(kc == 0),
                                             stop=(kc == F_CH - 1 and not with_bias))
                    if with_bias:
                        for j in range(2):
                            n = 2 * npair + j
                            nc.tensor.matmul(ts[j], wb[0:1, hsl],
                                             xones[:, 512 * n : 512 * (n + 1)],
                                             start=False, stop=True)
                    for j in range(2):
                        drains(half, j, ts[j])

            def qk_proj(w_sb, wb, o1, o2, npair):
                cs = [[None, None], [None, None]]

                def drains(half, j, ps):
                    c = tmp.tile([128, 512], bf16, tag="rope", name="c")
                    nc.scalar.copy(c, ps)
                    cs[half][j] = c

                wproj(w_sb, wb, npair, drains)
                for j in range(2):
                    n = 2 * npair + j
                    nsl = slice(512 * n, 512 * (n + 1))
                    c1, c2 = cs[0][j], cs[1][j]
                    t1 = tmp.tile([128, 512], bf16, tag="rope", name="t1")
                    t2 = tmp.tile([128, 512], bf16, tag="rope", name="t2")
                    t3 = tmp.tile([128, 512], bf16, tag="rope", name="t3")
                    t4 = tmp.tile([128, 512], bf16, tag="rope", name="t4")
                    nc.vector.tensor_mul(t1, c1, cosw[:, nsl])
                    nc.vector.tensor_mul(t2, c2, sinw[:, nsl])
                    nc.vector.tensor_mul(t3, c2, cosw[:, nsl])
                    nc.vector.tensor_mul(t4, c1, sinw[:, nsl])
                    nc.vector.tensor_sub(o1[:, nsl], t1, t2)
                    nc.vector.tensor_add(o2[:, nsl], t3, t4)

            def v_proj(npair):
                def drains(half, j, ps):
                    n = 2 * npair + j
                    nc.vector.tensor_copy(vT[half][:, 512 * n : 512 * (n + 1)], ps)

                wproj(wv, wvb, npair, drains)

            def v_transpose():
                # one strided xbar transpose per head: [64, 2048] ->
                # [(128x16), 64] with the 16 s-chunks 128 cols apart
                for half in range(2):
                    for hh in range(2):
                        hl = 2 * half + hh
                        out_v = v_sb[hl][:, :].rearrange("p (t c) -> p t c", c=128)[:, :, 0:64]
                        nc.sync.dma_start_transpose(
                            out=out_v, in_=vT[half][64 * hh : 64 * (hh + 1), :])

            # ---------------- attention -------------------------------
            at_tiles = {}  # (m, kk) -> [at_p0, at_p1]

            def scores_exp(m):
                for kk in range(4 * m + 4):
                    t = kk - 4 * m
                    lo = max(0, t) * 128
                    ksl = slice(128 * kk, 128 * (kk + 1))
                    qsl = slice(512 * m + lo, 512 * (m + 1))
                    sps_l = [psSps.tile([128, 1024], fp32, tag="sps", name=f"sps{p}")
                             for p in range(2)]
                    # wave order: all 4 heads' k1 matmuls (4 concurrent
                    # row-groups), then the k2 accumulate wave
                    for qq, kx in ((q1, k1), (q2, k2)):
                        for p in range(2):
                            for hh in range(2):
                                h = 2 * p + hh
                                hp = slice(32 * h, 32 * (h + 1))
                                osl = slice(512 * hh + lo, 512 * hh + 512)
                                nc.tensor.matmul(sps_l[p][:, osl], kx[hp, ksl], qq[hp, qsl],
                                                 start=(kx is k1), stop=(kx is k2),
                                                 tile_position=(32 * h, 0))
                    atp = []
                    for p in range(2):
                        at = attn_pool.tile([128, 1024], bf16, tag="attn", name="at")
                        sps_v = sps_l[p][:, :].rearrange("a (h q) -> a h q", h=2)[:, :, lo:512]
                        at_v = at[:, :].rearrange("a (h q) -> a h q", h=2)[:, :, lo:512]
                        nc.scalar.activation(out=at_v, in_=sps_v, func=EXP, scale=0.125)
                        if t >= 0:
                            dv = at[:, :].rearrange("a (h q) -> a h q", h=2)[:, :, 128 * t : 128 * (t + 1)]
                            nc.vector.tensor_tensor(dv, dv, maskt, op=MULT)
                        atp.append(at)
                    at_tiles[(m, kk)] = atp

            def finalize(pvt, h, m):
                msl = slice(512 * m, 512 * (m + 1))
                cix, r0 = h // 2, 64 * (h % 2)
                s65 = fin.tile([65, 512], fp32, tag="s65", name="s65")
                nc.vector.tensor_copy(s65, pvt[0:65, :])
                rec0 = fin.tile([1, 512], fp32, tag="rec0", name="rec0")
                nc.sync.dma_start(out=rec0, in_=s65[64:65, :])
                rcp = fin.tile([1, 512], fp32, tag="rcp", name="rcp")
                nc.vector.reciprocal_approx_fast(rcp, rec0)
                rb = fin.tile([64, 512], fp32, tag="rb", name="rb")
                nc.gpsimd.partition_broadcast(rb, rcp[0:1, :])
                if r0 == 0:
                    nc.vector.tensor_tensor(aoT[cix][0:64, msl], s65[0:64, :], rb, op=MULT)
                else:
                    ao = fin.tile([64, 512], bf16, tag="ao", name="ao")
                    nc.vector.tensor_tensor(ao, s65[0:64, :], rb, op=MULT)
                    nc.sync.dma_start(out=aoT[cix][64:128, msl], in_=ao)

            def pv_fin(m):
                last = 4 * m + 3
                for p in range(2):
                    pv = [psPV.tile([65, 512], fp32, tag="pvT", name="pvT") for _ in range(2)]
                    for kk in range(4 * m + 4):
                        lo = max(0, kk - 4 * m) * 128
                        at = at_tiles[(m, kk)][p]
                        for hh in range(2):
                            h = 2 * p + hh
                            nc.tensor.matmul(
                                pv[hh][:, lo:512],
                                v_sb[h][:, 128 * kk : 128 * kk + 65],
                                at[:, 512 * hh + lo : 512 * hh + 512],
                                start=(kk == 0), stop=(kk == last))
                    for hh in range(2):
                        finalize(pv[hh], 2 * p + hh, m)

            # ---------------- output projection ------------------------
            def outproj(sc):
                scsl = slice(512 * sc, 512 * (sc + 1))
                for fo in range(F_CH):
                    fsl = slice(128 * fo, 128 * (fo + 1))
                    t = psProj.tile([128, 512], fp32, tag="proj", name="ow")
                    nc.tensor.matmul(t, wo[0][:, fsl], aoT[0][:, scsl], start=True, stop=False)
                    nc.tensor.matmul(t, wo[1][:, fsl], aoT[1][:, scsl], start=False, stop=True)
                    ow = fin.tile([128, 512], bf16, tag="ow", name="owd")
                    if fo % 2 == 0:
                        nc.vector.tensor_copy(ow, t)
                    else:
                        nc.scalar.copy(ow, t)
                    nc.sync.dma_start(out=outT_d[fsl, scsl], in_=ow)

            # ---------------- emission schedule ------------------------
            # exp-feeding chain first (ACT is the scarce engine); PV /
            # finalize / out-projection trail as PE filler.
            qk01_streamed()
            scores_exp(0)
            scores_exp(1)
            qk_proj(wq, wqb, q1, q2, 1)
            qk_proj(wk, wkb, k1, k2, 1)
            v_proj(0)
            v_proj(1)
            v_transpose()
            scores_exp(2)
            pv_fin(0)
            pv_fin(1)
            scores_exp(3)
            pv_fin(2)
            outproj(0)
            outproj(1)
            pv_fin(3)
            outproj(2)
            outproj(3)

    nc.compile()
    return nc


def _get_nc(with_bias):
    key = ("nc", with_bias)
    if key not in _CACHE:
        _CACHE[key] = _build_nc(with_bias)
    return _CACHE[key]


def _host_prep(x, positions, Wq, bq, Wk, bk, Wv, bv, Wo, bo):
    """Build the 8 per-core input maps."""
    ts = MAX_WAVELENGTH ** (2.0 * np.arange(HALF, dtype=np.float32) / D)  # [32]
    in_maps = []
    for c in range(NCORES):
        b, g = c // 4, c % 4
        heads = np.arange(4 * g, 4 * g + 4)
        cols_x1 = np.concatenate([64 * h + np.arange(32) for h in heads])
        cols_x2 = cols_x1 + 32
        perm = np.concatenate([cols_x1, cols_x2])
        vcols = np.concatenate([64 * h + np.arange(64) for h in heads])

        xT = np.empty((F + 1, S), dtype=BF16)
        xT[:F] = x[b].T.astype(BF16)
        xT[F] = 1.0

        wq_e = np.empty((F + 1, 256), dtype=BF16)
        wq_e[:F] = Wq[:, perm].astype(BF16)
        wq_e[F] = bq[perm].astype(BF16)
        wk_e = np.empty((F + 1, 256), dtype=BF16)
        wk_e[:F] = Wk[:, perm].astype(BF16)
        wk_e[F] = bk[perm].astype(BF16)
        wv_e = np.empty((F + 1, 256), dtype=BF16)
        wv_e[:F] = Wv[:, vcols].astype(BF16)
        wv_e[F] = bv[vcols].astype(BF16)

        pos = positions[b].astype(np.float32)  # [S]
        ang = pos[None, :] / ts[:, None]  # [32, S]
        cosw = np.tile(np.cos(ang), (4, 1)).astype(BF16)
        sinw = np.tile(np.sin(ang), (4, 1)).astype(BF16)

        ii = np.arange(128)
        mask = np.tile((ii[:, None] <= ii[None, :]).astype(BF16), (1, 2))

        in_maps.append({
            "xT": xT,
            "wq": wq_e,
            "wk": wk_e,
            "wv": wv_e,
            "wo": Wo[64 * heads[0] : 64 * heads[0] + 256, :].astype(BF16),
            "cosw": cosw,
            "sinw": sinw,
            "mask": np.ascontiguousarray(mask),
        })
    return in_maps


def kernel(x, positions, Wq, bq, Wk, bk, Wv, bv, Wo, bo):
    global LAST_RESULT
    from concourse.bass_utils import run_bass_kernel_spmd

    x = np.asarray(x, dtype=np.float32)
    positions = np.asarray(positions)
    args = [np.asarray(a, dtype=np.float32) for a in (Wq, bq, Wk, bk, Wv, bv, Wo, bo)]
    Wq, bq, Wk, bk, Wv, bv, Wo, bo = args

    with_bias = bool(np.any(bq) or np.any(bk) or np.any(bv))
    nc = _get_nc(with_bias)
    in_maps = _host_prep(x, positions, Wq, bq, Wk, bk, Wv, bv, Wo, bo)
    try:
        res = run_bass_kernel_spmd(nc, in_maps, core_ids=list(range(NCORES)))
    except ModuleNotFoundError:
        # axon NTFF profiling hook unavailable in this image; run untraced
        os.environ["BASS_NEVER_TRACE"] = "1"
        res = run_bass_kernel_spmd(nc, in_maps, core_ids=list(range(NCORES)))
    LAST_RESULT = res

    out = np.empty((B, S, F), dtype=np.float32)
    for b in range(B):
        acc = np.zeros((F, S), dtype=np.float32)
        for g in range(4):
            acc += res.results[4 * b + g]["outT"].astype(np.float32)
        out[b] = acc.T + bo[None, :]
    return out
